# revision 20
# baseline (speedup 1.0000x reference)
"""Trainium2 Bass kernel for a custom Jacobi-basis layer.

Math:
    t = tanh(x)                                  x: [B, I] f32
    J[b,i,k] = P_k^(1,1)(t[b,i])                 Jacobi polys, k = 0..8
    out[b,o] = sum_{i,k} J[b,i,k] * coeff[o,i,k] * weights[o,i]

Strategy (8 NeuronCores, data-parallel over batch):
  * Change of basis: J_k are degree-8 polynomials in t, so the contraction
    sum_k J_k r_k equals sum_d V_d r~_d for ANY basis {V_d} of P_8, with the
    change-of-basis folded into the host-prepared matmul operand r~.
    We pick a product basis computable with 4 ScalarE activations and
    4 VectorE ops, all fp16, no serial depth-8 recurrence:
        V1=t, V2=t^2 (Square), V3=t*V2, V4=(V2+b4)^2 (Square w/ bias),
        V5=V2*V3, V6=(V4+b6)*V2 (STT), V7=V3*V4, V8=(V4+b8)^2.
    The shifts b4/b6/b8 were grid-searched to condition the transform
    (monomials alone amplify fp16 rounding to ~1.9e-2; shifted: ~2.8e-3).
  * V0=1 contributes a per-output bias, applied as one K=2 matmul per
    b-tile with (bias_hi, bias_lo) fp16 rows reconstructing fp32 bias.
  * Per core: 128 fp16 matmuls [128x128]@[128x512] accumulate fp32 in PSUM
    over the 4096-long (i,d) contraction; PE is the roofline (~28us).
  * DMA: all transfers issued up-front with no dependency ladder; the sync
    queue's serial descriptor-gen (~0.6us each) staggers them in program
    order (xt chunk 0, r1, consts, xt rest, r2..r8) and the 16 DMA queues
    stay saturated instead of idling behind completion chains.
  * PE is HAM-warmed with memset-sourced junk matmuls before the real
    stream so the 128-matmul stream runs at 2.4 GHz throughout.
"""

import numpy as np

import concourse.mybir as mybir
import concourse.tile as tile
from concourse import bacc
from concourse.bass_utils import run_bass_kernel_spmd

ORDER = 8
B, I, O = 4096, 512, 512
NCORES = 8
BC = B // NCORES          # batch rows per core = 512
P = 128                   # partitions
NIC = I // P              # i-chunks = 4
BT = BC // P              # b-tiles per core = 4
FREE = NIC * BC           # free dim of basis planes = 2048

# basis shift constants (grid-searched for fp16 conditioning)
B4, B6, B8 = -0.64, -0.06, -0.24
N_WARM = 8                # junk HAM-warmup matmuls (bridge to first real MM)


def _jacobi_monomial():
    """MJ[k, d]: monomial coefficients of P_k^(1,1), k,d = 0..8 (float64)."""
    a = b = 1.0
    MJ = np.zeros((ORDER + 1, ORDER + 1))
    MJ[0, 0] = 1.0
    MJ[1, 1] = 0.5 * (a + b + 2)
    MJ[1, 0] = -0.5 * (a - b)
    for i in range(2, ORDER + 1):
        k1 = (2 * i + a + b) * (2 * i + a + b - 1) / (2 * i * (i + a + b))
        k2 = (2 * i + a + b - 1) * (a * a - b * b) / (
            2 * i * (i + a + b) * (2 * i + a + b - 2)
        )
        k3 = (i + a - 1) * (i + b - 1) * (2 * i + a + b) / (
            i * (i + a + b) * (2 * i + a + b - 2)
        )
        MJ[i, 1:] += k1 * MJ[i - 1, :-1]
        MJ[i, :] += k2 * MJ[i - 1, :]
        MJ[i, :] -= k3 * MJ[i - 2, :]
    return MJ


def _basis_transform():
    """S[k, d] with J_k = sum_d S[k,d] V_d (V_0 = 1)."""
    def pmul(a, b):
        return np.convolve(a, b)[: ORDER + 1]

    MV = np.zeros((ORDER + 1, ORDER + 1))
    MV[0, 0] = 1
    MV[1, 1] = 1
    MV[2, 2] = 1
    MV[3, 3] = 1
    v2, v3 = MV[2], MV[3]
    v4 = pmul(v2 + B4 * MV[0], v2 + B4 * MV[0])
    MV[4] = v4
    MV[5] = pmul(v2, v3)
    MV[6] = pmul(v4 + B6 * MV[0], v2)
    MV[7] = pmul(v3, v4)
    MV[8] = pmul(v4 + B8 * MV[0], v4 + B8 * MV[0])
    return _jacobi_monomial() @ np.linalg.inv(MV)


def _build_module():
    nc = bacc.Bacc("TRN2", num_devices=NCORES)
    f32 = mybir.dt.float32
    f16 = mybir.dt.float16

    # xt chunks: xt[ic, p, b] = x[core*BC + b, ic*128 + p], pre-cast f16
    # (tanh sensitivity (1-t^2) makes f16 input rounding negligible: 2.9e-3)
    xt_d = nc.dram_tensor("xt", [NIC, P, BC], f16, kind="ExternalInput")
    # r layout: [p, (d-1)*FREE + ic*O + o] = Ct[o, ic*128+p, d]
    r_d = nc.dram_tensor("r", [P, ORDER * FREE], f16, kind="ExternalInput")
    # consts rows (hi, lo): [ones(128) | bias(512)]
    consts_d = nc.dram_tensor("consts", [2, P + O], f16, kind="ExternalInput")
    # out layout: [p, bt*O + o] = output[core*BC + bt*128 + p, o]
    out_d = nc.dram_tensor("out", [P, BT * O], f32, kind="ExternalOutput")

    mult = mybir.AluOpType.mult
    add = mybir.AluOpType.add
    Act = mybir.ActivationFunctionType

    with tile.TileContext(nc) as tc:
        with (
            tc.tile_pool(name="p", bufs=1) as pool,
            tc.tile_pool(name="psum", bufs=1, space="PSUM") as pp,
        ):
            def chunk(ap, ic):
                return ap[:, ic * BC : (ic + 1) * BC]

            # GpSimd is the first engine free after the preamble; memsets
            # there unblock the PE warmup ~1.5us earlier than VectorE would.
            warm_t = pool.tile([P, P + O], f16, tag="warm")
            nc.gpsimd.memset(warm_t[:], 0.25)
            # per-partition bias columns for the Square activations
            bcol_t = pool.tile([P, 2], f32, tag="bcol")
            nc.gpsimd.memset(bcol_t[:, 0:1], B4)
            nc.gpsimd.memset(bcol_t[:, 1:2], B8)

            # DMAs: program order = sync-queue gen order = arrival order.
            # r3..r8 ride a depth-2 completion ladder: pacing the HBM burst
            # (instead of saturating all 16 queues for ~15us) while still
            # delivering each plane well before the PE reaches it.
            from concourse.tile_rust import add_dep_helper

            x_t = pool.tile([P, FREE], f16, tag="x")
            r_t = [
                pool.tile([P, FREE], f16, tag=f"r{k}", name=f"r{k}")
                for k in range(ORDER)
            ]
            const_t = pool.tile([2, P + O], f16, tag="consts")
            nc.sync.dma_start(chunk(x_t, 0), xt_d[0])
            d_r = [None] * ORDER
            d_r[0] = nc.sync.dma_start(r_t[0][:], r_d[:, 0:FREE])
            for ic in range(1, NIC):
                nc.sync.dma_start(chunk(x_t, ic), xt_d[ic])
            nc.sync.dma_start(const_t[:], consts_d[:])
            for k in range(1, ORDER):
                d_r[k] = nc.sync.dma_start(
                    r_t[k][:], r_d[:, k * FREE : (k + 1) * FREE]
                )
                add_dep_helper(
                    d_r[k].ins, d_r[k - 1].ins, reason="dma pacing"
                )

            # Basis planes, all fp16.
            t = pool.tile([P, FREE], f16, tag="t")
            sq = pool.tile([P, FREE], f16, tag="sq")
            v3 = pool.tile([P, FREE], f16, tag="v3")
            v4 = pool.tile([P, FREE], f16, tag="v4")
            v5 = pool.tile([P, FREE], f16, tag="v5")
            v6 = pool.tile([P, FREE], f16, tag="v6")
            v7 = pool.tile([P, FREE], f16, tag="v7")
            v8 = pool.tile([P, FREE], f16, tag="v8")
            for ic in range(NIC):
                nc.scalar.activation(chunk(t, ic), chunk(x_t, ic), Act.Tanh)
            for ic in range(NIC):
                nc.scalar.activation(chunk(sq, ic), chunk(t, ic), Act.Square)
            H = FREE // 2
            halves = (slice(0, H), slice(H, FREE))
            for h in halves:
                nc.vector.tensor_tensor(v3[:, h], t[:, h], sq[:, h], mult)
            for h in halves:
                nc.scalar.activation(
                    v4[:, h], sq[:, h], Act.Square, bias=bcol_t[:, 0:1]
                )
            for h in halves:
                nc.vector.tensor_tensor(v5[:, h], sq[:, h], v3[:, h], mult)
            for h in halves:
                nc.vector.scalar_tensor_tensor(
                    v6[:, h], v4[:, h], B6, sq[:, h], add, mult
                )
            for h in halves:
                nc.vector.tensor_tensor(v7[:, h], v3[:, h], v4[:, h], mult)
            for h in halves:
                nc.scalar.activation(
                    v8[:, h], v4[:, h], Act.Square, bias=bcol_t[:, 1:2]
                )
            planes = [t, sq, v3, v4, v5, v6, v7, v8]

            # PE: HAM warmup -> bias (K=2, hi+lo rows) -> 8 plane blocks.
            ps_warm = pp.tile([P, O], f32, tag="warm", name="ps_warm")
            for _ in range(N_WARM):
                nc.tensor.matmul(
                    ps_warm[:],
                    warm_t[:, 0:P],
                    warm_t[:, P : P + O],
                    start=True,
                    stop=True,
                )
            psums = [
                pp.tile([P, O], f32, tag=f"ps{bt}", name=f"ps{bt}")
                for bt in range(BT)
            ]
            out_t = pool.tile([P, BT * O], f32, tag="out")
            for k in range(ORDER):
                if k < ORDER - 1:
                    for ic in range(NIC):
                        for bt in range(BT):
                            col = ic * BC + bt * P
                            nc.tensor.matmul(
                                psums[bt][:],
                                planes[k][:, col : col + P],
                                r_t[k][:, ic * O : (ic + 1) * O],
                                start=k == 0 and ic == 0,
                                stop=False,
                            )
                        if k == 0 and ic == 0:
                            # bias rides between plane-1 ic blocks: keeps
                            # consts off the critical path AND buys the
                            # tanh chain a block of slack for t(ic1)
                            for bt in range(BT):
                                nc.tensor.matmul(
                                    psums[bt][:],
                                    const_t[:, 0:P],
                                    const_t[:, P : P + O],
                                    start=False,
                                    stop=False,
                                )
                else:
                    # last block: per b-tile so evictions/stores overlap the
                    # remaining matmuls
                    for bt in range(BT):
                        for ic in range(NIC):
                            col = ic * BC + bt * P
                            nc.tensor.matmul(
                                psums[bt][:],
                                planes[k][:, col : col + P],
                                r_t[k][:, ic * O : (ic + 1) * O],
                                start=False,
                                stop=ic == NIC - 1,
                            )
                        dst = out_t[:, bt * O : (bt + 1) * O]
                        if bt % 2 == 0:
                            nc.scalar.copy(dst, psums[bt][:])
                        else:
                            nc.vector.tensor_copy(dst, psums[bt][:])
                        nc.sync.dma_start(
                            out_d[:, bt * O : (bt + 1) * O],
                            out_t[:, bt * O : (bt + 1) * O],
                        )
    nc.compile()
    return nc


def _prep_operands(weights, coeff):
    """Host-side, input-independent preprocessing of the layer constants."""
    S = _basis_transform()
    Cw = coeff.astype(np.float64) * weights.astype(np.float64)[:, :, None]
    Ct = np.einsum("oik,kd->oid", Cw, S)
    bias = Ct[:, :, 0].sum(axis=1)                       # [O] float64
    r = np.empty((ORDER, P, FREE), dtype=np.float32)
    for d in range(1, ORDER + 1):
        tmp = Ct[:, :, d].T.astype(np.float32)           # [I, O]
        r[d - 1] = tmp.reshape(NIC, P, O).transpose(1, 0, 2).reshape(P, FREE)
    r = np.ascontiguousarray(
        r.transpose(1, 0, 2).reshape(P, ORDER * FREE)
    ).astype(np.float16)
    bias_hi = bias.astype(np.float16)
    bias_lo = (bias - bias_hi.astype(np.float64)).astype(np.float16)
    consts = np.ones((2, P + O), dtype=np.float16)
    consts[0, P:] = bias_hi
    consts[1, P:] = bias_lo
    return r, consts


def _prep_x(x):
    """Per-core [NIC, 128, BC] views: xt[ic, p, b] = x[core*BC+b, ic*128+p]."""
    shards = []
    x16 = x.astype(np.float16)
    for core in range(NCORES):
        xc = np.ascontiguousarray(x16[core * BC : (core + 1) * BC, :].T)
        shards.append(np.ascontiguousarray(xc.reshape(NIC, P, BC)))
    return shards


def _install_ntff_hook():
    """Register the NTFF profile hook that the image's boot skips (no
    antenv.axon_hooks module). Same ctypes ABI as trn_boot's
    _ntff_profile_via_ctypes. Only used for traced (profiling) runs."""
    import sys
    import types
    import ctypes
    import contextlib

    if "antenv.axon_hooks" in sys.modules:
        return
    mod = types.ModuleType("antenv.axon_hooks")
    state = {"hook": None}
    mod.set_axon_ntff_profile_hook = lambda h: state.__setitem__("hook", h)
    mod.get_axon_ntff_profile_hook = lambda: state["hook"]
    sys.modules["antenv.axon_hooks"] = mod
    import antenv

    antenv.axon_hooks = mod

    so_path = "/opt/axon/libaxon_pjrt.so"
    lib = ctypes.CDLL(so_path)
    if not hasattr(lib, "axon_start_nrt_profile"):
        return
    lib.axon_start_nrt_profile.argtypes = [
        ctypes.POINTER(ctypes.c_int64),
        ctypes.c_size_t,
    ]
    lib.axon_start_nrt_profile.restype = ctypes.c_int64
    lib.axon_stop_nrt_profile.argtypes = [ctypes.c_char_p]
    lib.axon_stop_nrt_profile.restype = ctypes.c_int64

    @contextlib.contextmanager
    def _hook(output_dir, device_ids):
        import jax

        jax.devices()
        if device_ids:
            ids = (ctypes.c_int64 * len(device_ids))(*device_ids)
            rc = lib.axon_start_nrt_profile(ids, len(device_ids))
        else:
            rc = lib.axon_start_nrt_profile(None, 0)
        if rc != 0:
            raise RuntimeError(f"axon_start_nrt_profile rc={rc}")
        try:
            yield
        finally:
            n = lib.axon_stop_nrt_profile(str(output_dir).encode())
            print(f"ntff profile: {n} file(s) written to {output_dir}")

    mod.set_axon_ntff_profile_hook(_hook)


_NC_CACHE = None


def _get_module():
    global _NC_CACHE
    if _NC_CACHE is None:
        _NC_CACHE = _build_module()
    return _NC_CACHE


def _run(x, weights, coeff, trace=False):
    nc = _get_module()
    r, consts = _prep_operands(weights, coeff)
    xs = _prep_x(np.asarray(x, dtype=np.float32))
    in_maps = [
        {"xt": xs[core], "r": r, "consts": consts} for core in range(NCORES)
    ]
    try:
        res = run_bass_kernel_spmd(
            nc, in_maps, core_ids=list(range(NCORES)), trace=trace
        )
    except Exception:
        res = run_bass_kernel_spmd(
            nc, in_maps, core_ids=list(range(NCORES)), trace=trace
        )
    out = np.concatenate(
        [
            res.results[core]["out"]
            .reshape(P, BT, O)
            .transpose(1, 0, 2)
            .reshape(BC, O)
            for core in range(NCORES)
        ],
        axis=0,
    )
    return out, res


def kernel(x, weights, coeff):
    out, _ = _run(x, weights, coeff, trace=False)
    return out


def kernel_traced(x, weights, coeff):
    _install_ntff_hook()
    out, res = _run(x, weights, coeff, trace=True)
    return out, res


# revision 22
# speedup vs baseline: 1.0043x; 1.0043x over previous
"""Trainium2 Bass kernel for a custom Jacobi-basis layer.

Math:
    t = tanh(x)                                  x: [B, I] f32
    J[b,i,k] = P_k^(1,1)(t[b,i])                 Jacobi polys, k = 0..8
    out[b,o] = sum_{i,k} J[b,i,k] * coeff[o,i,k] * weights[o,i]

Strategy (8 NeuronCores, data-parallel over batch):
  * Change of basis: J_k are degree-8 polynomials in t, so the contraction
    sum_k J_k r_k equals sum_d V_d r~_d for ANY basis {V_d} of P_8, with the
    change-of-basis folded into the host-prepared matmul operand r~.
    We pick a product basis computable with 4 ScalarE activations and
    4 VectorE ops, all fp16, no serial depth-8 recurrence:
        V1=t, V2=t^2 (Square), V3=t*V2, V4=(V2+b4)^2 (Square w/ bias),
        V5=V2*V3, V6=(V4+b6)*V2 (STT), V7=V3*V4, V8=(V4+b8)^2.
    The shifts b4/b6/b8 were grid-searched to condition the transform
    (monomials alone amplify fp16 rounding to ~1.9e-2; shifted: ~2.8e-3).
  * V0=1 contributes a per-output bias, applied as one K=2 matmul per
    b-tile with (bias_hi, bias_lo) fp16 rows reconstructing fp32 bias.
  * Per core: 128 fp16 matmuls [128x128]@[128x512] accumulate fp32 in PSUM
    over the 4096-long (i,d) contraction; PE is the roofline (~28us).
  * DMA: all transfers issued up-front with no dependency ladder; the sync
    queue's serial descriptor-gen (~0.6us each) staggers them in program
    order (xt chunk 0, r1, consts, xt rest, r2..r8) and the 16 DMA queues
    stay saturated instead of idling behind completion chains.
  * PE is HAM-warmed with memset-sourced junk matmuls before the real
    stream so the 128-matmul stream runs at 2.4 GHz throughout.
"""

import numpy as np

import concourse.mybir as mybir
import concourse.tile as tile
from concourse import bacc
from concourse.bass_utils import run_bass_kernel_spmd

ORDER = 8
B, I, O = 4096, 512, 512
NCORES = 8
BC = B // NCORES          # batch rows per core = 512
P = 128                   # partitions
NIC = I // P              # i-chunks = 4
BT = BC // P              # b-tiles per core = 4
FREE = NIC * BC           # free dim of basis planes = 2048

# basis shift constants (grid-searched for fp16 conditioning)
B4, B6, B8 = -0.64, -0.06, -0.24
N_WARM = 8                # junk HAM-warmup matmuls (bridge to first real MM)


def _jacobi_monomial():
    """MJ[k, d]: monomial coefficients of P_k^(1,1), k,d = 0..8 (float64)."""
    a = b = 1.0
    MJ = np.zeros((ORDER + 1, ORDER + 1))
    MJ[0, 0] = 1.0
    MJ[1, 1] = 0.5 * (a + b + 2)
    MJ[1, 0] = -0.5 * (a - b)
    for i in range(2, ORDER + 1):
        k1 = (2 * i + a + b) * (2 * i + a + b - 1) / (2 * i * (i + a + b))
        k2 = (2 * i + a + b - 1) * (a * a - b * b) / (
            2 * i * (i + a + b) * (2 * i + a + b - 2)
        )
        k3 = (i + a - 1) * (i + b - 1) * (2 * i + a + b) / (
            i * (i + a + b) * (2 * i + a + b - 2)
        )
        MJ[i, 1:] += k1 * MJ[i - 1, :-1]
        MJ[i, :] += k2 * MJ[i - 1, :]
        MJ[i, :] -= k3 * MJ[i - 2, :]
    return MJ


def _basis_transform():
    """S[k, d] with J_k = sum_d S[k,d] V_d (V_0 = 1)."""
    def pmul(a, b):
        return np.convolve(a, b)[: ORDER + 1]

    MV = np.zeros((ORDER + 1, ORDER + 1))
    MV[0, 0] = 1
    MV[1, 1] = 1
    MV[2, 2] = 1
    MV[3, 3] = 1
    v2, v3 = MV[2], MV[3]
    v4 = pmul(v2 + B4 * MV[0], v2 + B4 * MV[0])
    MV[4] = v4
    MV[5] = pmul(v2, v3)
    MV[6] = pmul(v4 + B6 * MV[0], v2)
    MV[7] = pmul(v3, v4)
    MV[8] = pmul(v4 + B8 * MV[0], v4 + B8 * MV[0])
    return _jacobi_monomial() @ np.linalg.inv(MV)


def _build_module():
    nc = bacc.Bacc("TRN2", num_devices=NCORES)
    f32 = mybir.dt.float32
    f16 = mybir.dt.float16

    # xt chunks: xt[ic, p, b] = x[core*BC + b, ic*128 + p], pre-cast f16
    # (tanh sensitivity (1-t^2) makes f16 input rounding negligible: 2.9e-3)
    xt_d = nc.dram_tensor("xt", [NIC, P, BC], f16, kind="ExternalInput")
    # r layout: [p, (d-1)*FREE + ic*O + o] = Ct[o, ic*128+p, d]
    r_d = nc.dram_tensor("r", [P, ORDER * FREE], f16, kind="ExternalInput")
    # consts rows (hi, lo): [ones(128) | bias(512)]
    consts_d = nc.dram_tensor("consts", [2, P + O], f16, kind="ExternalInput")
    # out layout: [p, bt*O + o] = output[core*BC + bt*128 + p, o]
    out_d = nc.dram_tensor("out", [P, BT * O], f32, kind="ExternalOutput")

    mult = mybir.AluOpType.mult
    add = mybir.AluOpType.add
    Act = mybir.ActivationFunctionType

    with tile.TileContext(nc) as tc:
        with (
            tc.tile_pool(name="p", bufs=1) as pool,
            tc.tile_pool(name="psum", bufs=1, space="PSUM") as pp,
        ):
            def chunk(ap, ic):
                return ap[:, ic * BC : (ic + 1) * BC]

            # GpSimd is the first engine free after the preamble; memsets
            # there unblock the PE warmup ~1.5us earlier than VectorE would.
            warm_t = pool.tile([P, P + O], f16, tag="warm")
            nc.gpsimd.memset(warm_t[:], 0.25)
            # per-partition bias columns for the Square activations
            bcol_t = pool.tile([P, 2], f32, tag="bcol")
            nc.gpsimd.memset(bcol_t[:, 0:1], B4)
            nc.gpsimd.memset(bcol_t[:, 1:2], B8)

            # DMAs: program order = sync-queue gen order = arrival order.
            # r3..r8 ride a depth-2 completion ladder: pacing the HBM burst
            # (instead of saturating all 16 queues for ~15us) while still
            # delivering each plane well before the PE reaches it.
            from concourse.tile_rust import add_dep_helper

            x_t = pool.tile([P, FREE], f16, tag="x")
            r_t = [
                pool.tile([P, FREE], f16, tag=f"r{k}", name=f"r{k}")
                for k in range(ORDER)
            ]
            const_t = pool.tile([2, P + O], f16, tag="consts")
            nc.sync.dma_start(chunk(x_t, 0), xt_d[0])
            d_r = [None] * ORDER
            d_r[0] = nc.sync.dma_start(r_t[0][:], r_d[:, 0:FREE])
            for ic in range(1, NIC):
                nc.sync.dma_start(chunk(x_t, ic), xt_d[ic])
            nc.sync.dma_start(const_t[:], consts_d[:])
            # r2..r8 as half-plane transfers on a depth-2 completion ladder:
            # ~0.25MB per chain in flight, one half every ~1.5us — matches
            # the PE's 3.46us/plane consumption without saturating HBM
            # (full saturation provokes a mid-stream PE down-clock).
            HF = FREE // 2
            d_h = [None, None]
            d_h[0] = nc.sync.dma_start(
                r_t[1][:, 0:HF], r_d[:, FREE : FREE + HF]
            )
            d_h[1] = nc.sync.dma_start(
                r_t[1][:, HF:FREE], r_d[:, FREE + HF : 2 * FREE]
            )
            for k in range(2, ORDER):
                for h in range(2):
                    d = nc.sync.dma_start(
                        r_t[k][:, h * HF : (h + 1) * HF],
                        r_d[:, k * FREE + h * HF : k * FREE + (h + 1) * HF],
                    )
                    add_dep_helper(
                        d.ins, d_h[h].ins, reason="dma pacing"
                    )
                    d_h[h] = d

            # Basis planes, all fp16.
            t = pool.tile([P, FREE], f16, tag="t")
            sq = pool.tile([P, FREE], f16, tag="sq")
            v3 = pool.tile([P, FREE], f16, tag="v3")
            v4 = pool.tile([P, FREE], f16, tag="v4")
            v5 = pool.tile([P, FREE], f16, tag="v5")
            v6 = pool.tile([P, FREE], f16, tag="v6")
            v7 = pool.tile([P, FREE], f16, tag="v7")
            v8 = pool.tile([P, FREE], f16, tag="v8")
            for ic in range(NIC):
                nc.scalar.activation(chunk(t, ic), chunk(x_t, ic), Act.Tanh)
            for ic in range(NIC):
                nc.scalar.activation(chunk(sq, ic), chunk(t, ic), Act.Square)
            H = FREE // 2
            halves = (slice(0, H), slice(H, FREE))
            for h in halves:
                nc.vector.tensor_tensor(v3[:, h], t[:, h], sq[:, h], mult)
            for h in halves:
                nc.scalar.activation(
                    v4[:, h], sq[:, h], Act.Square, bias=bcol_t[:, 0:1]
                )
            for h in halves:
                nc.vector.tensor_tensor(v5[:, h], sq[:, h], v3[:, h], mult)
            for h in halves:
                nc.vector.scalar_tensor_tensor(
                    v6[:, h], v4[:, h], B6, sq[:, h], add, mult
                )
            for h in halves:
                nc.vector.tensor_tensor(v7[:, h], v3[:, h], v4[:, h], mult)
            for h in halves:
                nc.scalar.activation(
                    v8[:, h], v4[:, h], Act.Square, bias=bcol_t[:, 1:2]
                )
            planes = [t, sq, v3, v4, v5, v6, v7, v8]

            # PE: HAM warmup -> bias (K=2, hi+lo rows) -> 8 plane blocks.
            ps_warm = pp.tile([P, O], f32, tag="warm", name="ps_warm")
            for _ in range(N_WARM):
                nc.tensor.matmul(
                    ps_warm[:],
                    warm_t[:, 0:P],
                    warm_t[:, P : P + O],
                    start=True,
                    stop=True,
                )
            psums = [
                pp.tile([P, O], f32, tag=f"ps{bt}", name=f"ps{bt}")
                for bt in range(BT)
            ]
            out_t = pool.tile([P, BT * O], f32, tag="out")
            for k in range(ORDER):
                if k < ORDER - 1:
                    for ic in range(NIC):
                        for bt in range(BT):
                            col = ic * BC + bt * P
                            nc.tensor.matmul(
                                psums[bt][:],
                                planes[k][:, col : col + P],
                                r_t[k][:, ic * O : (ic + 1) * O],
                                start=k == 0 and ic == 0,
                                stop=False,
                            )
                    if k == 0:
                        # bias rides behind plane 1 so consts stays off the
                        # stream-start critical path
                        for bt in range(BT):
                            nc.tensor.matmul(
                                psums[bt][:],
                                const_t[:, 0:P],
                                const_t[:, P : P + O],
                                start=False,
                                stop=False,
                            )
                else:
                    # last block: per b-tile so evictions/stores overlap the
                    # remaining matmuls
                    for bt in range(BT):
                        for ic in range(NIC):
                            col = ic * BC + bt * P
                            nc.tensor.matmul(
                                psums[bt][:],
                                planes[k][:, col : col + P],
                                r_t[k][:, ic * O : (ic + 1) * O],
                                start=False,
                                stop=ic == NIC - 1,
                            )
                        dst = out_t[:, bt * O : (bt + 1) * O]
                        if bt % 2 == 0:
                            nc.scalar.copy(dst, psums[bt][:])
                        else:
                            nc.vector.tensor_copy(dst, psums[bt][:])
                        nc.sync.dma_start(
                            out_d[:, bt * O : (bt + 1) * O],
                            out_t[:, bt * O : (bt + 1) * O],
                        )
    nc.compile()
    return nc


def _prep_operands(weights, coeff):
    """Host-side, input-independent preprocessing of the layer constants."""
    S = _basis_transform()
    Cw = coeff.astype(np.float64) * weights.astype(np.float64)[:, :, None]
    Ct = np.einsum("oik,kd->oid", Cw, S)
    bias = Ct[:, :, 0].sum(axis=1)                       # [O] float64
    r = np.empty((ORDER, P, FREE), dtype=np.float32)
    for d in range(1, ORDER + 1):
        tmp = Ct[:, :, d].T.astype(np.float32)           # [I, O]
        r[d - 1] = tmp.reshape(NIC, P, O).transpose(1, 0, 2).reshape(P, FREE)
    r = np.ascontiguousarray(
        r.transpose(1, 0, 2).reshape(P, ORDER * FREE)
    ).astype(np.float16)
    bias_hi = bias.astype(np.float16)
    bias_lo = (bias - bias_hi.astype(np.float64)).astype(np.float16)
    consts = np.ones((2, P + O), dtype=np.float16)
    consts[0, P:] = bias_hi
    consts[1, P:] = bias_lo
    return r, consts


def _prep_x(x):
    """Per-core [NIC, 128, BC] views: xt[ic, p, b] = x[core*BC+b, ic*128+p]."""
    shards = []
    x16 = x.astype(np.float16)
    for core in range(NCORES):
        xc = np.ascontiguousarray(x16[core * BC : (core + 1) * BC, :].T)
        shards.append(np.ascontiguousarray(xc.reshape(NIC, P, BC)))
    return shards


def _install_ntff_hook():
    """Register the NTFF profile hook that the image's boot skips (no
    antenv.axon_hooks module). Same ctypes ABI as trn_boot's
    _ntff_profile_via_ctypes. Only used for traced (profiling) runs."""
    import sys
    import types
    import ctypes
    import contextlib

    if "antenv.axon_hooks" in sys.modules:
        return
    mod = types.ModuleType("antenv.axon_hooks")
    state = {"hook": None}
    mod.set_axon_ntff_profile_hook = lambda h: state.__setitem__("hook", h)
    mod.get_axon_ntff_profile_hook = lambda: state["hook"]
    sys.modules["antenv.axon_hooks"] = mod
    import antenv

    antenv.axon_hooks = mod

    so_path = "/opt/axon/libaxon_pjrt.so"
    lib = ctypes.CDLL(so_path)
    if not hasattr(lib, "axon_start_nrt_profile"):
        return
    lib.axon_start_nrt_profile.argtypes = [
        ctypes.POINTER(ctypes.c_int64),
        ctypes.c_size_t,
    ]
    lib.axon_start_nrt_profile.restype = ctypes.c_int64
    lib.axon_stop_nrt_profile.argtypes = [ctypes.c_char_p]
    lib.axon_stop_nrt_profile.restype = ctypes.c_int64

    @contextlib.contextmanager
    def _hook(output_dir, device_ids):
        import jax

        jax.devices()
        if device_ids:
            ids = (ctypes.c_int64 * len(device_ids))(*device_ids)
            rc = lib.axon_start_nrt_profile(ids, len(device_ids))
        else:
            rc = lib.axon_start_nrt_profile(None, 0)
        if rc != 0:
            raise RuntimeError(f"axon_start_nrt_profile rc={rc}")
        try:
            yield
        finally:
            n = lib.axon_stop_nrt_profile(str(output_dir).encode())
            print(f"ntff profile: {n} file(s) written to {output_dir}")

    mod.set_axon_ntff_profile_hook(_hook)


_NC_CACHE = None


def _get_module():
    global _NC_CACHE
    if _NC_CACHE is None:
        _NC_CACHE = _build_module()
    return _NC_CACHE


def _run(x, weights, coeff, trace=False):
    nc = _get_module()
    r, consts = _prep_operands(weights, coeff)
    xs = _prep_x(np.asarray(x, dtype=np.float32))
    in_maps = [
        {"xt": xs[core], "r": r, "consts": consts} for core in range(NCORES)
    ]
    try:
        res = run_bass_kernel_spmd(
            nc, in_maps, core_ids=list(range(NCORES)), trace=trace
        )
    except Exception:
        res = run_bass_kernel_spmd(
            nc, in_maps, core_ids=list(range(NCORES)), trace=trace
        )
    out = np.concatenate(
        [
            res.results[core]["out"]
            .reshape(P, BT, O)
            .transpose(1, 0, 2)
            .reshape(BC, O)
            for core in range(NCORES)
        ],
        axis=0,
    )
    return out, res


def kernel(x, weights, coeff):
    out, _ = _run(x, weights, coeff, trace=False)
    return out


def kernel_traced(x, weights, coeff):
    _install_ntff_hook()
    out, res = _run(x, weights, coeff, trace=True)
    return out, res


# revision 25
# speedup vs baseline: 1.0487x; 1.0442x over previous
"""Trainium2 Bass kernel for a custom Jacobi-basis layer.

Math:
    t = tanh(x)                                  x: [B, I] f32
    J[b,i,k] = P_k^(1,1)(t[b,i])                 Jacobi polys, k = 0..8
    out[b,o] = sum_{i,k} J[b,i,k] * coeff[o,i,k] * weights[o,i]

Strategy (8 NeuronCores, data-parallel over batch):
  * Change of basis: J_k are degree-8 polynomials in t, so the contraction
    sum_k J_k r_k equals sum_d V_d r~_d for ANY basis {V_d} of P_8, with the
    change-of-basis folded into the host-prepared matmul operand r~.
    We pick a product basis computable with 4 ScalarE activations and
    4 VectorE ops, all fp16, no serial depth-8 recurrence:
        V1=t, V2=t^2 (Square), V3=t*V2, V4=(V2+b4)^2 (Square w/ bias),
        V5=V2*V3, V6=(V4+b6)*V2 (STT), V7=V3*V4, V8=(V4+b8)^2.
    The shifts b4/b6/b8 were grid-searched to condition the transform
    (monomials alone amplify fp16 rounding to ~1.9e-2; shifted: ~2.8e-3).
  * V0=1 contributes a per-output bias, applied as one K=2 matmul per
    b-tile with (bias_hi, bias_lo) fp16 rows reconstructing fp32 bias.
  * Per core: 128 fp16 matmuls [128x128]@[128x512] accumulate fp32 in PSUM
    over the 4096-long (i,d) contraction; PE is the roofline (~28us).
  * DMA: all transfers issued up-front with no dependency ladder; the sync
    queue's serial descriptor-gen (~0.6us each) staggers them in program
    order (xt chunk 0, r1, consts, xt rest, r2..r8) and the 16 DMA queues
    stay saturated instead of idling behind completion chains.
  * PE is HAM-warmed with memset-sourced junk matmuls before the real
    stream so the 128-matmul stream runs at 2.4 GHz throughout.
"""

import numpy as np

import concourse.mybir as mybir
import concourse.tile as tile
from concourse import bacc
from concourse.bass_utils import run_bass_kernel_spmd

ORDER = 8
B, I, O = 4096, 512, 512
NCORES = 8
BC = B // NCORES          # batch rows per core = 512
P = 128                   # partitions
NIC = I // P              # i-chunks = 4
BT = BC // P              # b-tiles per core = 4
FREE = NIC * BC           # free dim of basis planes = 2048

# basis shift constants (grid-searched for fp16 conditioning)
B4, B6, B8 = -0.64, -0.06, -0.24
N_WARM = 8                # junk HAM-warmup matmuls (bridge to first real MM)


def _jacobi_monomial():
    """MJ[k, d]: monomial coefficients of P_k^(1,1), k,d = 0..8 (float64)."""
    a = b = 1.0
    MJ = np.zeros((ORDER + 1, ORDER + 1))
    MJ[0, 0] = 1.0
    MJ[1, 1] = 0.5 * (a + b + 2)
    MJ[1, 0] = -0.5 * (a - b)
    for i in range(2, ORDER + 1):
        k1 = (2 * i + a + b) * (2 * i + a + b - 1) / (2 * i * (i + a + b))
        k2 = (2 * i + a + b - 1) * (a * a - b * b) / (
            2 * i * (i + a + b) * (2 * i + a + b - 2)
        )
        k3 = (i + a - 1) * (i + b - 1) * (2 * i + a + b) / (
            i * (i + a + b) * (2 * i + a + b - 2)
        )
        MJ[i, 1:] += k1 * MJ[i - 1, :-1]
        MJ[i, :] += k2 * MJ[i - 1, :]
        MJ[i, :] -= k3 * MJ[i - 2, :]
    return MJ


def _basis_transform():
    """S[k, d] with J_k = sum_d S[k,d] V_d (V_0 = 1)."""
    def pmul(a, b):
        return np.convolve(a, b)[: ORDER + 1]

    MV = np.zeros((ORDER + 1, ORDER + 1))
    MV[0, 0] = 1
    MV[1, 1] = 1
    MV[2, 2] = 1
    MV[3, 3] = 1
    v2, v3 = MV[2], MV[3]
    v4 = pmul(v2 + B4 * MV[0], v2 + B4 * MV[0])
    MV[4] = v4
    MV[5] = pmul(v2, v3)
    MV[6] = pmul(v4 + B6 * MV[0], v2)
    MV[7] = pmul(v3, v4)
    MV[8] = pmul(v4 + B8 * MV[0], v4 + B8 * MV[0])
    return _jacobi_monomial() @ np.linalg.inv(MV)


def _build_module():
    nc = bacc.Bacc("TRN2", num_devices=NCORES)
    f32 = mybir.dt.float32
    f16 = mybir.dt.float16

    # xt chunks: xt[ic, p, b] = x[core*BC + b, ic*128 + p], pre-cast f16
    # (tanh sensitivity (1-t^2) makes f16 input rounding negligible: 2.9e-3)
    xt_d = nc.dram_tensor("xt", [NIC, P, BC], f16, kind="ExternalInput")
    # r layout: [p, (d-1)*FREE + ic*O + o] = Ct[o, ic*128+p, d]
    r_d = nc.dram_tensor("r", [P, ORDER * FREE], f16, kind="ExternalInput")
    # consts rows (hi, lo): [ones(128) | bias(512)]
    consts_d = nc.dram_tensor("consts", [2, P + O], f16, kind="ExternalInput")
    # out layout: [p, bt*O + o] = output[core*BC + bt*128 + p, o]
    out_d = nc.dram_tensor("out", [P, BT * O], f32, kind="ExternalOutput")

    mult = mybir.AluOpType.mult
    add = mybir.AluOpType.add
    Act = mybir.ActivationFunctionType

    with tile.TileContext(nc) as tc:
        with (
            tc.tile_pool(name="p", bufs=1) as pool,
            tc.tile_pool(name="psum", bufs=1, space="PSUM") as pp,
        ):
            def chunk(ap, ic):
                return ap[:, ic * BC : (ic + 1) * BC]

            # GpSimd is the first engine free after the preamble; memsets
            # there unblock the PE warmup ~1.5us earlier than VectorE would.
            warm_t = pool.tile([P, P + O], f16, tag="warm")
            nc.gpsimd.memset(warm_t[:], 0.25)
            # per-partition bias columns for the Square activations
            bcol_t = pool.tile([P, 2], f32, tag="bcol")
            nc.gpsimd.memset(bcol_t[:, 0:1], B4)
            nc.gpsimd.memset(bcol_t[:, 1:2], B8)

            # DMAs: program order = sync-queue gen order = arrival order.
            # r3..r8 ride a depth-2 completion ladder: pacing the HBM burst
            # (instead of saturating all 16 queues for ~15us) while still
            # delivering each plane well before the PE reaches it.
            from concourse.tile_rust import add_dep_helper

            x_t = pool.tile([P, FREE], f16, tag="x")
            r_t = [
                pool.tile([P, FREE], f16, tag=f"r{k}", name=f"r{k}")
                for k in range(ORDER)
            ]
            const_t = pool.tile([2, P + O], f16, tag="consts")
            nc.sync.dma_start(chunk(x_t, 0), xt_d[0])
            # r1 split: the 128KB ic0 slice completes ~1.4us before the
            # full plane would, unblocking the first real matmuls
            nc.sync.dma_start(r_t[0][:, 0:O], r_d[:, 0:O])
            nc.sync.dma_start(chunk(x_t, 1), xt_d[1])
            d_r1b = nc.sync.dma_start(r_t[0][:, O:FREE], r_d[:, O:FREE])
            for ic in range(2, NIC):
                nc.sync.dma_start(chunk(x_t, ic), xt_d[ic])
            nc.sync.dma_start(const_t[:], consts_d[:])
            # r2..r8 as half-plane transfers on a depth-2 completion ladder:
            # ~0.25MB per chain in flight, one half every ~1.5us — matches
            # the PE's 3.46us/plane consumption without saturating HBM
            # (full saturation provokes a mid-stream PE down-clock).
            HF = FREE // 2
            d_h = [d_r1b, d_r1b]
            for k in range(1, ORDER):
                for h in range(2):
                    d = nc.sync.dma_start(
                        r_t[k][:, h * HF : (h + 1) * HF],
                        r_d[:, k * FREE + h * HF : k * FREE + (h + 1) * HF],
                    )
                    add_dep_helper(
                        d.ins, d_h[h].ins, reason="dma pacing"
                    )
                    d_h[h] = d

            # Basis planes, all fp16.
            t = pool.tile([P, FREE], f16, tag="t")
            sq = pool.tile([P, FREE], f16, tag="sq")
            v3 = pool.tile([P, FREE], f16, tag="v3")
            v4 = pool.tile([P, FREE], f16, tag="v4")
            v5 = pool.tile([P, FREE], f16, tag="v5")
            v6 = pool.tile([P, FREE], f16, tag="v6")
            v7 = pool.tile([P, FREE], f16, tag="v7")
            v8 = pool.tile([P, FREE], f16, tag="v8")
            for ic in range(NIC):
                nc.scalar.activation(chunk(t, ic), chunk(x_t, ic), Act.Tanh)
            for ic in range(NIC):
                nc.scalar.activation(chunk(sq, ic), chunk(t, ic), Act.Square)
            H = FREE // 2
            halves = (slice(0, H), slice(H, FREE))
            for h in halves:
                nc.vector.tensor_tensor(v3[:, h], t[:, h], sq[:, h], mult)
            for h in halves:
                nc.scalar.activation(
                    v4[:, h], sq[:, h], Act.Square, bias=bcol_t[:, 0:1]
                )
            for h in halves:
                nc.vector.tensor_tensor(v5[:, h], sq[:, h], v3[:, h], mult)
            for h in halves:
                nc.vector.scalar_tensor_tensor(
                    v6[:, h], v4[:, h], B6, sq[:, h], add, mult
                )
            for h in halves:
                nc.vector.tensor_tensor(v7[:, h], v3[:, h], v4[:, h], mult)
            for h in halves:
                nc.scalar.activation(
                    v8[:, h], v4[:, h], Act.Square, bias=bcol_t[:, 1:2]
                )
            planes = [t, sq, v3, v4, v5, v6, v7, v8]

            # PE: HAM warmup -> bias (K=2, hi+lo rows) -> 8 plane blocks.
            ps_warm = pp.tile([P, O], f32, tag="warm", name="ps_warm")
            for _ in range(N_WARM):
                nc.tensor.matmul(
                    ps_warm[:],
                    warm_t[:, 0:P],
                    warm_t[:, P : P + O],
                    start=True,
                    stop=True,
                )
            psums = [
                pp.tile([P, O], f32, tag=f"ps{bt}", name=f"ps{bt}")
                for bt in range(BT)
            ]
            out_t = pool.tile([P, BT * O], f32, tag="out")
            for k in range(ORDER):
                if k < ORDER - 1:
                    for ic in range(NIC):
                        for bt in range(BT):
                            col = ic * BC + bt * P
                            nc.tensor.matmul(
                                psums[bt][:],
                                planes[k][:, col : col + P],
                                r_t[k][:, ic * O : (ic + 1) * O],
                                start=k == 0 and ic == 0,
                                stop=False,
                            )
                    if k == 0:
                        # bias rides behind plane 1 so consts stays off the
                        # stream-start critical path
                        for bt in range(BT):
                            nc.tensor.matmul(
                                psums[bt][:],
                                const_t[:, 0:P],
                                const_t[:, P : P + O],
                                start=False,
                                stop=False,
                            )
                else:
                    # last block: per b-tile so evictions/stores overlap the
                    # remaining matmuls
                    for bt in range(BT):
                        for ic in range(NIC):
                            col = ic * BC + bt * P
                            nc.tensor.matmul(
                                psums[bt][:],
                                planes[k][:, col : col + P],
                                r_t[k][:, ic * O : (ic + 1) * O],
                                start=False,
                                stop=ic == NIC - 1,
                            )
                        dst = out_t[:, bt * O : (bt + 1) * O]
                        if bt % 2 == 0:
                            nc.vector.tensor_copy(dst, psums[bt][:])
                        else:
                            # ScalarE is PSUM-adjacent (570 vs 690ns) —
                            # give it the last b-tile's exposed evict
                            nc.scalar.copy(dst, psums[bt][:])
                        nc.sync.dma_start(
                            out_d[:, bt * O : (bt + 1) * O],
                            out_t[:, bt * O : (bt + 1) * O],
                        )
    nc.compile()
    return nc


def _prep_operands(weights, coeff):
    """Host-side, input-independent preprocessing of the layer constants."""
    S = _basis_transform()
    Cw = coeff.astype(np.float64) * weights.astype(np.float64)[:, :, None]
    Ct = np.einsum("oik,kd->oid", Cw, S)
    bias = Ct[:, :, 0].sum(axis=1)                       # [O] float64
    r = np.empty((ORDER, P, FREE), dtype=np.float32)
    for d in range(1, ORDER + 1):
        tmp = Ct[:, :, d].T.astype(np.float32)           # [I, O]
        r[d - 1] = tmp.reshape(NIC, P, O).transpose(1, 0, 2).reshape(P, FREE)
    r = np.ascontiguousarray(
        r.transpose(1, 0, 2).reshape(P, ORDER * FREE)
    ).astype(np.float16)
    bias_hi = bias.astype(np.float16)
    bias_lo = (bias - bias_hi.astype(np.float64)).astype(np.float16)
    consts = np.ones((2, P + O), dtype=np.float16)
    consts[0, P:] = bias_hi
    consts[1, P:] = bias_lo
    return r, consts


def _prep_x(x):
    """Per-core [NIC, 128, BC] views: xt[ic, p, b] = x[core*BC+b, ic*128+p]."""
    shards = []
    x16 = x.astype(np.float16)
    for core in range(NCORES):
        xc = np.ascontiguousarray(x16[core * BC : (core + 1) * BC, :].T)
        shards.append(np.ascontiguousarray(xc.reshape(NIC, P, BC)))
    return shards


def _install_ntff_hook():
    """Register the NTFF profile hook that the image's boot skips (no
    antenv.axon_hooks module). Same ctypes ABI as trn_boot's
    _ntff_profile_via_ctypes. Only used for traced (profiling) runs."""
    import sys
    import types
    import ctypes
    import contextlib

    if "antenv.axon_hooks" in sys.modules:
        return
    mod = types.ModuleType("antenv.axon_hooks")
    state = {"hook": None}
    mod.set_axon_ntff_profile_hook = lambda h: state.__setitem__("hook", h)
    mod.get_axon_ntff_profile_hook = lambda: state["hook"]
    sys.modules["antenv.axon_hooks"] = mod
    import antenv

    antenv.axon_hooks = mod

    so_path = "/opt/axon/libaxon_pjrt.so"
    lib = ctypes.CDLL(so_path)
    if not hasattr(lib, "axon_start_nrt_profile"):
        return
    lib.axon_start_nrt_profile.argtypes = [
        ctypes.POINTER(ctypes.c_int64),
        ctypes.c_size_t,
    ]
    lib.axon_start_nrt_profile.restype = ctypes.c_int64
    lib.axon_stop_nrt_profile.argtypes = [ctypes.c_char_p]
    lib.axon_stop_nrt_profile.restype = ctypes.c_int64

    @contextlib.contextmanager
    def _hook(output_dir, device_ids):
        import jax

        jax.devices()
        if device_ids:
            ids = (ctypes.c_int64 * len(device_ids))(*device_ids)
            rc = lib.axon_start_nrt_profile(ids, len(device_ids))
        else:
            rc = lib.axon_start_nrt_profile(None, 0)
        if rc != 0:
            raise RuntimeError(f"axon_start_nrt_profile rc={rc}")
        try:
            yield
        finally:
            n = lib.axon_stop_nrt_profile(str(output_dir).encode())
            print(f"ntff profile: {n} file(s) written to {output_dir}")

    mod.set_axon_ntff_profile_hook(_hook)


_NC_CACHE = None


def _get_module():
    global _NC_CACHE
    if _NC_CACHE is None:
        _NC_CACHE = _build_module()
    return _NC_CACHE


def _run(x, weights, coeff, trace=False):
    nc = _get_module()
    r, consts = _prep_operands(weights, coeff)
    xs = _prep_x(np.asarray(x, dtype=np.float32))
    in_maps = [
        {"xt": xs[core], "r": r, "consts": consts} for core in range(NCORES)
    ]
    try:
        res = run_bass_kernel_spmd(
            nc, in_maps, core_ids=list(range(NCORES)), trace=trace
        )
    except Exception:
        res = run_bass_kernel_spmd(
            nc, in_maps, core_ids=list(range(NCORES)), trace=trace
        )
    out = np.concatenate(
        [
            res.results[core]["out"]
            .reshape(P, BT, O)
            .transpose(1, 0, 2)
            .reshape(BC, O)
            for core in range(NCORES)
        ],
        axis=0,
    )
    return out, res


def kernel(x, weights, coeff):
    out, _ = _run(x, weights, coeff, trace=False)
    return out


def kernel_traced(x, weights, coeff):
    _install_ntff_hook()
    out, res = _run(x, weights, coeff, trace=True)
    return out, res
